# revision 2
# baseline (speedup 1.0000x reference)
"""Trainium2 Bass kernel for nn_DiagonalTransfer.

Math: out[i, j] = logsumexp_k(A[i, k] + xx[k, j]) with A = diag(d) (dense,
zeros off-diagonal). This collapses to

    out[i, j] = log( sum_k W[i, k] * exp(xx[k, j]) ),   W = ones + diag(e^d - 1)

i.e. a pointwise exp, a tiny stationary GEMM over the 64 states, and a
pointwise log.

Layout: xx is [64, B]. Shard B across 8 cores: per-core [64, BC] slice,
converted to fp16 host-side and REINTERPRETED as [128, BC/2]: partition p
holds state p//2, column half p%2 — dense 128-partition DMAs. The GEMM
weight is the parity-interleaved W2[p,q] = g*W[p//2, q//2] * (p%2 == q%2).

v2 engine split (engine budgets per core per pass, DC=262144 device cols):
  - exp via SINGLE-term Schraudolph on DVE: int16 bits = rne(x*1477.32 +
    15360) bitcast fp16 is ~e^x*(1+sawtooth(+/-3%)); one 4x-mode
    tensor_scalar pass (~72 us). The +/-3% is absorbed: validated fp64
    numpy sim of this exact pipeline gives max rel err 0.0046 (ACT path) /
    0.0108 (DVE log path) vs the 2e-2 gate.
  - PE: ONE fp16 matmul term per column (vs 2 in v1): DC cols @ ~1 col/cyc
    warm 2.4 GHz ~= 110-130 us.
  - Ln drain of PSUM is split by span: 11/16 of spans drain via ACT Ln
    (1 elem/cyc @ 1.2 GHz, ~158 us), 5/16 via DVE bits-linear log
    (out = fp32(bits32(y))*ln2/2^23 + c, one 1x tensor_scalar from PSUM,
    ~90 us) — neither engine alone can cover DC (ACT full pass = 230 us >
    ~187 us DMA floor).
  - all DMAs on the SP HWDGE ring (measured ~740 GB/s/core combined).
"""

import numpy as np

N = 64
B = 4_194_304
NCORES = 8
BC = B // NCORES            # 524288 original columns per core
DC = BC // 2                # 262144 device columns in the [128, DC] view

SPAN = 8192                 # device columns per DMA/pipeline span
PSUM_TF = 1024              # PSUM tile (2 banks); 8 per span
MM_FREE = 512               # matmul free dim (one fp32 PSUM bank)
PSBUFS = 4
DRAIN_DEFER = 3
XBUFS, EBUFS, OBUFS = 4, 3, 3

# 'A' span -> ACT Ln drain; 'D' span -> DVE bits-log drain (11A/5D)
PAT = "AADAADAADAADAADA"

SCH_A = 1477.3197           # 1024 * log2(e)
SCH_C = 15360.0             # bias: x=0 -> fp16 bits of 1.0
GAIN = 0.9625               # weight gain centering the Schraudolph sawtooth
SA = 1.0029138517481218     # ACT Ln pre-scale (centers residual log bias)
K2 = 8.262958294867817e-08  # ln2 / 2^23
C2 = -87.99591819947364     # bits-log bias (incl. -ln GAIN and centering)

_prog_cache = {}

# This walrus build rejects instructions carrying more than one sync wait
# ("Too many sync wait commands" in CoreV*GenImpl::setupSyncWait), but Tile
# attaches multi-sem waits to instructions (and its kernel-tail drain waits
# on every outstanding semaphore at once). Move excess waits onto preceding
# NoOp carriers on the same engine — the sequencer blocks on each in order,
# which is equivalent to waiting them jointly.
_MAX_WAITS = 1


def _split_waits(nc):
    import bass_rust
    import concourse.mybir as mybir

    for fn in nc.m.functions:
        for blk in fn.blocks:
            insts = blk.instructions
            i = 0
            while i < len(insts):
                ins = insts[i]
                si = ins.sync_info
                if si is not None and len(si.on_wait) > _MAX_WAITS:
                    waits = list(si.on_wait)
                    keep = waits[-_MAX_WAITS:]
                    for w in waits[:-_MAX_WAITS]:
                        d = bass_rust.InstNoOp(
                            name=nc.get_next_instruction_name(), ins=[], outs=[]
                        )
                        d.engine = ins.engine
                        d.sync_info = mybir.SyncInfo(on_wait=[w], on_update=[])
                        nc.register_instruction(d)
                        insts.insert(i, d)
                        i += 1
                    si.on_wait = keep
                i += 1


def _build_program(reps=1):
    import concourse.bass as bass
    import concourse.mybir as mybir
    from concourse.tile import TileContext

    f32 = mybir.dt.float32
    f16 = mybir.dt.float16
    i16 = mybir.dt.int16
    i32 = mybir.dt.int32
    A = mybir.AluOpType
    Ln = mybir.ActivationFunctionType.Ln
    nspan = DC // SPAN

    nc = bass.Bass()
    xx_d = nc.declare_dram_parameter("xx", [128, DC], f16, isOutput=False)
    w_d = nc.declare_dram_parameter("w0", [128, 128], f16, isOutput=False)
    out_d = nc.declare_dram_parameter("out", [128, DC], f16, isOutput=True)

    with TileContext(nc) as tc:
        with (
            tc.tile_pool(name="wpool", bufs=1) as wpool,
            tc.tile_pool(name="xpool", bufs=XBUFS) as xpool,
            tc.tile_pool(name="epool", bufs=EBUFS) as epool,
            tc.tile_pool(name="opool", bufs=OBUFS) as opool,
            tc.tile_pool(name="pspool", bufs=PSBUFS, space="PSUM") as pspool,
        ):
            w_sb = wpool.tile([128, 128], f16, name="w0")
            nc.sync.dma_start(w_sb[:], w_d[:])

            pend_drain = []
            pend_store = []

            def emit_drain():
                ps, out_ap, kind = pend_drain.pop(0)
                if kind == "A":
                    nc.scalar.activation(out_ap, ps[:], Ln, scale=float(SA))
                else:
                    nc.vector.tensor_scalar(
                        out_ap, ps[:].bitcast(i32),
                        float(K2), float(C2), A.mult, A.add,
                    )

            def emit_store(pend):
                t, o_t = pend
                nc.sync.dma_start(out_d[:, t * SPAN:(t + 1) * SPAN], o_t[:])

            spans = [t for _ in range(reps) for t in range(nspan)]
            for idx, t in enumerate(spans):
                kind = PAT[idx % len(PAT)]
                x_t = xpool.tile([128, SPAN], f16)
                nc.sync.dma_start(x_t[:], xx_d[:, t * SPAN:(t + 1) * SPAN])
                o_t = opool.tile([128, SPAN], f16, name="o_t")
                e_t = epool.tile([128, SPAN], i16, name="e_t")
                for h in range(2):
                    sl = slice(h * (SPAN // 2), (h + 1) * (SPAN // 2))
                    nc.vector.tensor_scalar(
                        e_t[:, sl], x_t[:, sl], SCH_A, SCH_C, A.mult, A.add,
                    )
                e_mm = e_t[:].bitcast(f16)
                for q in range(SPAN // PSUM_TF):
                    ps = pspool.tile([128, PSUM_TF], f32)
                    for k in range(PSUM_TF // MM_FREE):
                        off = q * PSUM_TF + k * MM_FREE
                        nc.tensor.matmul(
                            ps[:, k * MM_FREE:(k + 1) * MM_FREE],
                            w_sb[:],
                            e_mm[:, off:off + MM_FREE],
                            start=True,
                            stop=True,
                        )
                    if len(pend_drain) >= DRAIN_DEFER:
                        emit_drain()
                    pend_drain.append(
                        (ps, o_t[:, q * PSUM_TF:(q + 1) * PSUM_TF], kind)
                    )
                if pend_store:
                    emit_store(pend_store.pop(0))
                pend_store.append((t, o_t))
            while pend_drain:
                emit_drain()
            for pend in pend_store:
                emit_store(pend)
    _split_waits(nc)
    return nc


def _weights(diag):
    d64 = np.asarray(diag, dtype=np.float64)
    W = np.full((N, N), GAIN, dtype=np.float64)
    W[np.arange(N), np.arange(N)] = GAIN * np.exp(d64)
    # Parity-interleaved blockdiag for the [128, DC] reinterpretation:
    # partition p = (state p//2, half p%2); halves don't mix.
    W2 = np.zeros((128, 128), dtype=np.float64)
    idx = np.arange(128)
    for par in (0, 1):
        rows = idx[idx % 2 == par]
        W2[np.ix_(rows, rows)] = W[np.ix_(rows // 2, rows // 2)]
    return {"w0": W2.astype(np.float16)}


def _in_maps(xx, diag):
    ws = _weights(diag)
    xx16 = np.ascontiguousarray(np.asarray(xx, dtype=np.float32)).astype(
        np.float16
    )
    return [
        {
            "xx": np.ascontiguousarray(xx16[:, c * BC:(c + 1) * BC]).reshape(
                128, DC
            ),
            **ws,
        }
        for c in range(NCORES)
    ]


def _run(xx, diag, **kw):
    from concourse.bass_utils import run_bass_kernel_spmd

    assert np.asarray(xx).shape == (N, B)
    if "prog" not in _prog_cache:
        _prog_cache["prog"] = _build_program()
    nc = _prog_cache["prog"]

    in_maps = _in_maps(xx, diag)
    res = run_bass_kernel_spmd(nc, in_maps, list(range(NCORES)), **kw)
    out = np.concatenate(
        [
            np.asarray(res.results[c]["out"])
            .astype(np.float32)
            .reshape(N, BC)
            for c in range(NCORES)
        ],
        axis=1,
    )
    return out, res


def kernel(xx, diag):
    out, _ = _run(xx, diag)
    return out.astype(np.float32, copy=False)


# revision 5
# speedup vs baseline: 1.0519x; 1.0519x over previous
"""Trainium2 Bass kernel for nn_DiagonalTransfer.

Math: out[i, j] = logsumexp_k(A[i, k] + xx[k, j]) with A = diag(d) (dense,
zeros off-diagonal). This collapses to

    out[i, j] = log( sum_k W[i, k] * exp(xx[k, j]) ),   W = ones + diag(e^d - 1)

i.e. a pointwise exp, a tiny stationary GEMM over the 64 states, and a
pointwise log.

Layout: xx is [64, B]. Shard B across 8 cores: per-core [64, BC] slice,
converted to fp16 host-side and REINTERPRETED as [128, BC/2]: partition p
holds state p//2, column half p%2 — dense 128-partition DMAs. The GEMM
weight is the parity-interleaved W2[p,q] = g*W[p//2, q//2] * (p%2 == q%2).

v2 engine split (engine budgets per core per pass, DC=262144 device cols):
  - exp via SINGLE-term Schraudolph on DVE: int16 bits = rne(x*1477.32 +
    15360) bitcast fp16 is ~e^x*(1+sawtooth(+/-3%)); one 4x-mode
    tensor_scalar pass (~72 us). The +/-3% is absorbed: validated fp64
    numpy sim of this exact pipeline gives max rel err 0.0046 (ACT path) /
    0.0108 (DVE log path) vs the 2e-2 gate.
  - PE: ONE fp16 matmul term per column (vs 2 in v1): DC cols @ ~1 col/cyc
    warm 2.4 GHz ~= 110-130 us.
  - Ln drain of PSUM is split by span: 11/16 of spans drain via ACT Ln
    (1 elem/cyc @ 1.2 GHz, ~158 us), 5/16 via DVE bits-linear log
    (out = fp32(bits32(y))*ln2/2^23 + c, one 1x tensor_scalar from PSUM,
    ~90 us) — neither engine alone can cover DC (ACT full pass = 230 us >
    ~187 us DMA floor).
  - all DMAs on the SP HWDGE ring (measured ~740 GB/s/core combined).
"""

import numpy as np

N = 64
B = 4_194_304
NCORES = 8
BC = B // NCORES            # 524288 original columns per core
DC = BC // 2                # 262144 device columns in the [128, DC] view

SPAN = 8192                 # device columns per DMA/pipeline span
PSUM_TF = 1024              # PSUM tile (2 banks); 8 per span
MM_FREE = 512               # matmul free dim (one fp32 PSUM bank)
PSBUFS = 4
DRAIN_DEFER = 3
XBUFS, EBUFS, OBUFS = 4, 3, 3

# Per-span PSUM-tile drain assignment: 'A' -> ACT Ln, 'D' -> DVE bits-log.
# Interleaved at tile level so both engines drain every span (span-level
# splits serialize: ACT idles during DVE spans and vice versa). Alternating
# 6A2D / 5A3D gives 11A/5D per span pair (DVE share 0.3125).
PATS = ("AADAADAA", "ADAADAAD")

SCH_A = 1477.3197           # 1024 * log2(e)
SCH_C = 15360.0             # bias: x=0 -> fp16 bits of 1.0
GAIN = 0.9625               # weight gain centering the Schraudolph sawtooth
SA = 1.0029138517481218     # ACT Ln pre-scale (centers residual log bias)
K2 = 8.262958294867817e-08  # ln2 / 2^23
C2 = -87.99591819947364     # bits-log bias (incl. -ln GAIN and centering)

_prog_cache = {}

# This walrus build rejects instructions carrying more than one sync wait
# ("Too many sync wait commands" in CoreV*GenImpl::setupSyncWait), but Tile
# attaches multi-sem waits to instructions (and its kernel-tail drain waits
# on every outstanding semaphore at once). Move excess waits onto preceding
# NoOp carriers on the same engine — the sequencer blocks on each in order,
# which is equivalent to waiting them jointly.
_MAX_WAITS = 1


def _split_waits(nc):
    import bass_rust
    import concourse.mybir as mybir

    for fn in nc.m.functions:
        for blk in fn.blocks:
            insts = blk.instructions
            i = 0
            while i < len(insts):
                ins = insts[i]
                si = ins.sync_info
                if si is not None and len(si.on_wait) > _MAX_WAITS:
                    waits = list(si.on_wait)
                    keep = waits[-_MAX_WAITS:]
                    for w in waits[:-_MAX_WAITS]:
                        d = bass_rust.InstNoOp(
                            name=nc.get_next_instruction_name(), ins=[], outs=[]
                        )
                        d.engine = ins.engine
                        d.sync_info = mybir.SyncInfo(on_wait=[w], on_update=[])
                        nc.register_instruction(d)
                        insts.insert(i, d)
                        i += 1
                    si.on_wait = keep
                i += 1


def _build_program(reps=1):
    import concourse.bass as bass
    import concourse.mybir as mybir
    from concourse.tile import TileContext

    f32 = mybir.dt.float32
    f16 = mybir.dt.float16
    i16 = mybir.dt.int16
    i32 = mybir.dt.int32
    A = mybir.AluOpType
    Ln = mybir.ActivationFunctionType.Ln
    nspan = DC // SPAN

    nc = bass.Bass()
    xx_d = nc.declare_dram_parameter("xx", [128, DC], f16, isOutput=False)
    w_d = nc.declare_dram_parameter("w0", [128, 128], f16, isOutput=False)
    out_d = nc.declare_dram_parameter("out", [128, DC], f16, isOutput=True)

    with TileContext(nc) as tc:
        with (
            tc.tile_pool(name="wpool", bufs=1) as wpool,
            tc.tile_pool(name="xpool", bufs=XBUFS) as xpool,
            tc.tile_pool(name="epool", bufs=EBUFS) as epool,
            tc.tile_pool(name="opool", bufs=OBUFS) as opool,
            tc.tile_pool(name="pspool", bufs=PSBUFS, space="PSUM") as pspool,
        ):
            w_sb = wpool.tile([128, 128], f16, name="w0")
            nc.sync.dma_start(w_sb[:], w_d[:])

            pend_drain = []
            pend_store = []

            def emit_drain():
                ps, out_ap, kind = pend_drain.pop(0)
                if kind == "A":
                    nc.scalar.activation(out_ap, ps[:], Ln, scale=float(SA))
                else:
                    nc.vector.tensor_scalar(
                        out_ap, ps[:].bitcast(i32),
                        float(K2), float(C2), A.mult, A.add,
                    )

            def emit_store(pend):
                t, o_t = pend
                nc.sync.dma_start(out_d[:, t * SPAN:(t + 1) * SPAN], o_t[:])

            spans = [t for _ in range(reps) for t in range(nspan)]
            for idx, t in enumerate(spans):
                pat = PATS[idx % 2]
                x_t = xpool.tile([128, SPAN], f16)
                nc.sync.dma_start(x_t[:], xx_d[:, t * SPAN:(t + 1) * SPAN])
                o_t = opool.tile([128, SPAN], f16, name="o_t")
                e_t = epool.tile([128, SPAN], i16, name="e_t")
                for h in range(2):
                    sl = slice(h * (SPAN // 2), (h + 1) * (SPAN // 2))
                    nc.vector.tensor_scalar(
                        e_t[:, sl], x_t[:, sl], SCH_A, SCH_C, A.mult, A.add,
                    )
                e_mm = e_t[:].bitcast(f16)
                for q in range(SPAN // PSUM_TF):
                    ps = pspool.tile([128, PSUM_TF], f32)
                    for k in range(PSUM_TF // MM_FREE):
                        off = q * PSUM_TF + k * MM_FREE
                        nc.tensor.matmul(
                            ps[:, k * MM_FREE:(k + 1) * MM_FREE],
                            w_sb[:],
                            e_mm[:, off:off + MM_FREE],
                            start=True,
                            stop=True,
                        )
                    if len(pend_drain) >= DRAIN_DEFER:
                        emit_drain()
                    pend_drain.append(
                        (ps, o_t[:, q * PSUM_TF:(q + 1) * PSUM_TF], pat[q])
                    )
                if pend_store:
                    emit_store(pend_store.pop(0))
                pend_store.append((t, o_t))
            while pend_drain:
                emit_drain()
            for pend in pend_store:
                emit_store(pend)
    _split_waits(nc)
    return nc


def _weights(diag):
    d64 = np.asarray(diag, dtype=np.float64)
    W = np.full((N, N), GAIN, dtype=np.float64)
    W[np.arange(N), np.arange(N)] = GAIN * np.exp(d64)
    # Parity-interleaved blockdiag for the [128, DC] reinterpretation:
    # partition p = (state p//2, half p%2); halves don't mix.
    W2 = np.zeros((128, 128), dtype=np.float64)
    idx = np.arange(128)
    for par in (0, 1):
        rows = idx[idx % 2 == par]
        W2[np.ix_(rows, rows)] = W[np.ix_(rows // 2, rows // 2)]
    return {"w0": W2.astype(np.float16)}


def _in_maps(xx, diag):
    ws = _weights(diag)
    xx16 = np.ascontiguousarray(np.asarray(xx, dtype=np.float32)).astype(
        np.float16
    )
    return [
        {
            "xx": np.ascontiguousarray(xx16[:, c * BC:(c + 1) * BC]).reshape(
                128, DC
            ),
            **ws,
        }
        for c in range(NCORES)
    ]


def _run(xx, diag, **kw):
    from concourse.bass_utils import run_bass_kernel_spmd

    assert np.asarray(xx).shape == (N, B)
    if "prog" not in _prog_cache:
        _prog_cache["prog"] = _build_program()
    nc = _prog_cache["prog"]

    in_maps = _in_maps(xx, diag)
    res = run_bass_kernel_spmd(nc, in_maps, list(range(NCORES)), **kw)
    out = np.concatenate(
        [
            np.asarray(res.results[c]["out"])
            .astype(np.float32)
            .reshape(N, BC)
            for c in range(NCORES)
        ],
        axis=1,
    )
    return out, res


def kernel(xx, diag):
    out, _ = _run(xx, diag)
    return out.astype(np.float32, copy=False)
